# revision 25
# baseline (speedup 1.0000x reference)
"""Multi-head self-attention (B=4, T=2048, D=1024, H=16) on 8 TRN2 NeuronCores.

Sharding: tensor-parallel over heads. Core c owns heads (2c, 2c+1):
  - W_Q/W_K/W_V rows [128c, 128c+128) -> per-core q/k/v of shape [T*B, 128]
  - causal attention for its 2 heads
  - partial output projection through W_O columns [128c, 128c+128)
Host sums the 8 partial outputs (the row-parallel W_O reduction).

Per-core schedule (batch-fused so engines stay dense):
  for b in 0..3:
    phase1(b): q/k/v projections for batch b, one 512-token chunk at a
      time (x streamed chunk-major, ONE dma per chunk).
    attn(b), per 512-q chunk: per 128-k-tile: h-packed score MMs (heads
      at array rows 0-63/64-127, concurrent) -> diag-mask add (DVE) ->
      exp (ACT, bf16 out) -> AV MMs accumulating [64 v-dims | ones] so
      row 64 is the softmax denominator. Diagonal k-tiles are column-
      sliced so fully-masked blocks are never computed.
      Chunk tail: 1/denom via DVE reciprocal_approx_fast (no ACT table
      swaps), GpSimd partition_broadcast, DVE mul into hoT, then the
      output projection + DMA for that chunk's 4 token-tiles.
  The dataflow scheduler overlaps attn(b) with phase1(b+1), keeping the
  PE dense (HAM stays at K=8/8) while ACT runs the exp stream.
"""

import os
import sys

import numpy as np

if "/opt/trn_rl_repo" not in sys.path:
    sys.path.insert(0, "/opt/trn_rl_repo")

import ml_dtypes

B, T, D, NH, DH = 4, 2048, 1024, 16, 64
NT = B * T          # 8192 tokens
MT = D // 128       # 8 model-dim tiles
NCH = NT // 512     # 16 token chunks
N_CORES = 8

_cache = {}


def _build_nc():
    from contextlib import ExitStack

    import concourse.mybir as mybir
    import concourse.tile as tile
    from concourse import bacc

    BF = mybir.dt.bfloat16
    F32 = mybir.dt.float32
    EXP = mybir.ActivationFunctionType.Exp
    LN = mybir.ActivationFunctionType.Ln

    nc = bacc.Bacc("TRN2", target_bir_lowering=False, debug=False)

    # x chunk-major: [chunk, partition, mt, col]
    xT_d = nc.dram_tensor("xT", [NCH, 128, MT, 512], BF, kind="ExternalInput")
    wq_d = nc.dram_tensor("wqT", [128, MT, 128], BF, kind="ExternalInput")
    wk_d = nc.dram_tensor("wkT", [128, MT, 128], BF, kind="ExternalInput")
    wv_d = nc.dram_tensor("wvT", [128, MT, 128], BF, kind="ExternalInput")
    wo_d = nc.dram_tensor("woT", [128, D], BF, kind="ExternalInput")
    cm_d = nc.dram_tensor("cmask", [128, 2, 128], BF, kind="ExternalInput")
    out_d = nc.dram_tensor("out", [NT, D], BF, kind="ExternalOutput")

    with tile.TileContext(nc) as tc, ExitStack() as ctx:
        pers = ctx.enter_context(tc.tile_pool(name="pers", bufs=1))
        wq = pers.tile([128, MT, 128], BF)
        wk = pers.tile([128, MT, 128], BF)
        wv = pers.tile([128, MT, 128], BF)
        wo = pers.tile([128, D], BF)
        cmask = pers.tile([128, 2, 128], BF)

        P = ctx.enter_context
        xpool = P(tc.tile_pool(name="xc", bufs=4))
        qtp = P(tc.tile_pool(name="qt", bufs=3))
        ktp = P(tc.tile_pool(name="kt", bufs=3))
        vbp = P(tc.tile_pool(name="vbp", bufs=3))
        hop = P(tc.tile_pool(name="hop", bufs=3))
        exp_pool = P(tc.tile_pool(name="exp", bufs=6))
        uhp = P(tc.tile_pool(name="uhp", bufs=3))
        denp = P(tc.tile_pool(name="den", bufs=2))
        lnp = P(tc.tile_pool(name="lnp", bufs=2))
        invp = P(tc.tile_pool(name="inv", bufs=2))
        bcast_p = P(tc.tile_pool(name="bcast", bufs=2))
        rvp = P(tc.tile_pool(name="rvp", bufs=4))
        osbp = P(tc.tile_pool(name="osb", bufs=4))
        p1p = P(tc.tile_pool(name="p1", bufs=1, space="PSUM"))
        popp = P(tc.tile_pool(name="pop", bufs=1, space="PSUM"))
        spp = P(tc.tile_pool(name="sp", bufs=2, space="PSUM"))
        avp = P(tc.tile_pool(name="avp", bufs=1, space="PSUM"))

        if True:
            state = {"first": True}
            qkv = {}

            def phase1(b):
                # ---- phase 1: q/k/v projections for batch b ----
                first = state["first"]
                qt = qtp.tile([128, T], BF, tag="qt", name="qt")
                kt = ktp.tile([128, T], BF, tag="kt", name="kt")
                vb = vbp.tile([128, 16, 130], BF, tag="vb", name="vb")
                qkv[b] = (qt, kt, vb)
                for lc in range(4):
                    c = 4 * b + lc
                    cs = slice(lc * 512, (lc + 1) * 512)
                    if first:
                        # wq first (needed by the very first MM), then the
                        # first x chunk, then the rest of the weights
                        nc.sync.dma_start(out=wq, in_=wq_d[:])
                    xc = xpool.tile([128, MT, 512], BF, tag="xc", name="xc")
                    if first:
                        nc.sync.dma_start(out=xc[:, 0:4, :], in_=xT_d[c, :, 0:4, :])
                        nc.sync.dma_start(out=xc[:, 4:8, :], in_=xT_d[c, :, 4:8, :])
                    else:
                        nc.sync.dma_start(out=xc, in_=xT_d[c])
                    if first:
                        nc.sync.dma_start(out=wk, in_=wk_d[:])
                        nc.sync.dma_start(out=wv, in_=wv_d[:])
                        nc.sync.dma_start(out=wo[:], in_=wo_d[:])
                        nc.sync.dma_start(out=cmask, in_=cm_d[:])
                        first = state["first"] = False
                    pq = p1p.tile([128, 512], F32, tag="p1", name="pq")
                    for mt in range(MT):
                        nc.tensor.matmul(pq, wq[:, mt, :], xc[:, mt, :],
                                         start=(mt == 0), stop=(mt == MT - 1))
                    nc.vector.tensor_copy(out=qt[:, cs], in_=pq)
                    pk = p1p.tile([128, 512], F32, tag="p1", name="pk")
                    for mt in range(MT):
                        nc.tensor.matmul(pk, wk[:, mt, :], xc[:, mt, :],
                                         start=(mt == 0), stop=(mt == MT - 1))
                    nc.vector.tensor_copy(out=kt[:, cs], in_=pk)
                    pv = p1p.tile([128, 4, 128], F32, tag="p1", name="pv")
                    for tt in range(4):
                        for mt in range(MT):
                            nc.tensor.matmul(pv[:, tt, :],
                                             xc[:, mt, tt * 128:(tt + 1) * 128],
                                             wv[:, mt, :],
                                             start=(mt == 0), stop=(mt == MT - 1))
                    for h in range(2):
                        nc.vector.tensor_copy(
                            out=vb[:, lc * 4:(lc + 1) * 4, 65 * h:65 * h + 64],
                            in_=pv[:, :, 64 * h:64 * h + 64])
                # softmax-denominator ones columns
                nc.vector.memset(vb[:, :, 64:65], 1.0)
                nc.vector.memset(vb[:, :, 129:130], 1.0)

            def attn(b):
                # ---- phase 2: causal attention + projection for batch b ----
                qt, kt, vb = qkv.pop(b)
                hoT = hop.tile([128, T], BF, tag="hoT", name="hoT")
                uh_g = {}
                # last batch: singleton tail groups so the final
                # normalization+projection drains earlier
                groups = ((0, 1), (2, 3)) if b < B - 1 else ((0, 1), (2,), (3,))
                for grp in groups:
                    deng = denp.tile([128, 512], F32, tag="deng",
                                     name="deng")
                    nc.vector.memset(deng, 1.0)
                    for gi, qc in enumerate(grp):
                        q0 = qc * 512
                        nk = 4 * qc + 4
                        pavs = [avp.tile([65, 512], F32, tag=f"pav{h}",
                                         name=f"pav{h}") for h in range(2)]
                        for kt_i in range(nk):
                            off = 128 * (kt_i - 4 * qc) if kt_i >= 4 * qc else 0
                            pss = spp.tile([128, 2, 512], F32, tag="pss",
                                           name="pss")
                            for h in range(2):
                                hp = 64 * h
                                nc.tensor.matmul(
                                    pss[:, h, off:512],
                                    kt[hp:hp + 64, kt_i * 128:(kt_i + 1) * 128],
                                    qt[hp:hp + 64, q0 + off:q0 + 512],
                                    start=True, stop=True)
                            ex = exp_pool.tile([128, 2, 512], BF, tag="ex",
                                               name="ex")
                            nc.scalar.activation(out=ex[:, :, off:512],
                                                 in_=pss[:, :, off:512],
                                                 func=EXP, scale=0.125)
                            if kt_i >= 4 * qc:
                                nc.vector.tensor_mul(ex[:, :, off:off + 128],
                                                     ex[:, :, off:off + 128],
                                                     cmask)
                            for h in range(2):
                                nc.tensor.matmul(
                                    pavs[h][:, off:512],
                                    vb[:, kt_i, 65 * h:65 * h + 65],
                                    ex[:, h, off:512],
                                    start=(kt_i == 0), stop=(kt_i == nk - 1))
                        # chunk tail: stash unnormalized AV rows in SBUF
                        # (frees PSUM fast); gather denominators into
                        # 32-aligned partitions of the group tile so one
                        # ln+exp ACT pass covers the whole group.
                        uh = uhp.tile([128, 2, 512], BF, tag="uh", name="uh")
                        uh_g[qc] = uh
                        for h in range(2):
                            nc.vector.tensor_copy(out=uh[0:64, h, :],
                                                  in_=pavs[h][0:64, :])
                            r = 64 * gi + 32 * h
                            nc.vector.tensor_copy(out=deng[r:r + 1, :],
                                                  in_=pavs[h][64:65, :])
                    # group tail: 1/denoms, broadcast, normalize, project
                    lnd = lnp.tile([128, 512], F32, tag="lnd", name="lnd")
                    nc.scalar.activation(out=lnd, in_=deng, func=LN)
                    invg = invp.tile([128, 512], BF, tag="invg", name="invg")
                    with nc.allow_low_precision(
                            reason="softmax 1/denom via exp(-ln d)"):
                        nc.scalar.activation(out=invg, in_=lnd, func=EXP,
                                             scale=-1.0)
                    for gi, qc2 in enumerate(grp):
                        q2 = qc2 * 512
                        for h in range(2):
                            r = 64 * gi + 32 * h
                            # partition_broadcast only honors base
                            # partition 0 on HW: stage the row there
                            rv = rvp.tile([1, 512], BF, tag="rv", name="rv")
                            nc.vector.tensor_copy(out=rv,
                                                  in_=invg[r:r + 1, :])
                            invb = bcast_p.tile([64, 512], BF,
                                                tag=f"invb{h}",
                                                name=f"invb{h}")
                            nc.gpsimd.partition_broadcast(invb, rv)
                            nc.vector.tensor_mul(
                                hoT[64 * h:64 * h + 64, q2:q2 + 512],
                                uh_g[qc2][0:64, h, :], invb)
                    for qc2 in grp:
                        for tt in range(4 * qc2, 4 * qc2 + 4):
                            osb = osbp.tile([128, D], BF, tag="osb",
                                            name="osb")
                            for oc in range(2):
                                po = popp.tile([128, 512], F32, tag="po",
                                               name="po")
                                nc.tensor.matmul(
                                    po, hoT[:, tt * 128:(tt + 1) * 128],
                                    wo[:, oc * 512:(oc + 1) * 512],
                                    start=True, stop=True)
                                nc.vector.tensor_copy(
                                    out=osb[:, oc * 512:(oc + 1) * 512],
                                    in_=po)
                            to = b * T + tt * 128
                            nc.sync.dma_start(out=out_d[to:to + 128, :],
                                              in_=osb)

            # Emit phase1(b+1) BEFORE attn(b): Tile's per-engine sem
            # waits are tick thresholds, so a consumer of a DVE write
            # waits for every earlier-priority DVE op — next-batch q/k/v
            # casts must precede the previous batch's attention tail or
            # the whole batch serializes on DVE order.
            for b in range(B):
                phase1(b)
                attn(b)
    # Compile with the activation-table chooser steered to the one set
    # that contains BOTH exp and ln, so the kernel needs a single
    # ACT_TABLE_LOAD (the default per-function choice alternates between
    # exp_and_others and natural_log, reloading tables at every switch).
    # Set ids stay valid: the dict keeps its size and insertion order,
    # only the membership used for selection is narrowed.
    import concourse.bacc as bacc_mod
    orig_tables = bacc_mod.get_activation_tables

    def _steered_tables(arch):
        tabs = orig_tables(arch)
        keep = "natural_log_exp_and_others"
        if keep in tabs:
            tabs = {name: (fns if name == keep else fns - tabs[keep])
                    for name, fns in tabs.items()}
        return tabs

    bacc_mod.get_activation_tables = _steered_tables
    try:
        nc.compile()
    finally:
        bacc_mod.get_activation_tables = orig_tables
    return nc


def _get_nc():
    if "nc" not in _cache:
        _cache["nc"] = _build_nc()
    return _cache["nc"]


def _bf(a):
    return np.ascontiguousarray(a, dtype=np.float32).astype(ml_dtypes.bfloat16)


def make_in_maps(x, W_Q, W_K, W_V, W_O):
    xT = _bf(x.reshape(NT, D).T)                      # [D, NT]
    # [chunk, partition, mt, col]
    xTc = np.ascontiguousarray(
        xT.reshape(MT, 128, NCH, 512).transpose(2, 1, 0, 3))
    cmask = np.ones((128, 2, 128), dtype=np.float32)
    for kp in range(128):
        cmask[kp, :, :kp] = 0.0
    cmask = cmask.astype(ml_dtypes.bfloat16)
    in_maps = []
    for c in range(N_CORES):
        rs = slice(c * 128, (c + 1) * 128)
        in_maps.append({
            "xT": xTc,
            "wqT": np.ascontiguousarray(
                _bf(W_Q[rs, :].T).reshape(MT, 128, 128).transpose(1, 0, 2)),
            "wkT": np.ascontiguousarray(
                _bf(W_K[rs, :].T).reshape(MT, 128, 128).transpose(1, 0, 2)),
            "wvT": np.ascontiguousarray(
                _bf(W_V[rs, :].T).reshape(MT, 128, 128).transpose(1, 0, 2)),
            "woT": _bf(W_O[:, rs].T),
            "cmask": cmask,
        })
    return in_maps


def _ensure_ntff_hook():
    """Install antenv.axon_hooks shim (missing in this image) so
    run_bass_kernel_spmd(trace=True) can capture NTFF profiles."""
    try:
        from antenv import axon_hooks  # noqa: F401
        return True
    except ImportError:
        pass
    try:
        import contextlib
        import ctypes
        import types

        import antenv

        so_path = "/opt/axon/libaxon_pjrt.so"
        lib = ctypes.CDLL(so_path)
        if not hasattr(lib, "axon_start_nrt_profile"):
            return False
        lib.axon_start_nrt_profile.argtypes = [
            ctypes.POINTER(ctypes.c_int64), ctypes.c_size_t]
        lib.axon_start_nrt_profile.restype = ctypes.c_int64
        lib.axon_stop_nrt_profile.argtypes = [ctypes.c_char_p]
        lib.axon_stop_nrt_profile.restype = ctypes.c_int64

        @contextlib.contextmanager
        def _hook(output_dir, device_ids):
            import jax
            jax.devices()
            if device_ids:
                ids = (ctypes.c_int64 * len(device_ids))(*device_ids)
                rc = lib.axon_start_nrt_profile(ids, len(device_ids))
            else:
                rc = lib.axon_start_nrt_profile(None, 0)
            if rc != 0:
                raise RuntimeError(f"axon_start_nrt_profile rc={rc}")
            try:
                yield
            finally:
                n = lib.axon_stop_nrt_profile(str(output_dir).encode())
                print(f"ntff profile: {n} file(s) -> {output_dir}",
                      file=sys.stderr)

        mod = types.ModuleType("antenv.axon_hooks")
        mod.get_axon_ntff_profile_hook = lambda: _hook
        mod.set_axon_ntff_profile_hook = lambda h: None
        sys.modules["antenv.axon_hooks"] = mod
        antenv.axon_hooks = mod
        return True
    except Exception as e:  # pragma: no cover
        print(f"ntff hook install failed: {e}", file=sys.stderr)
        return False


def run_on_cores(in_maps, trace=False, trace_all_cores=False):
    """Compile once, run on cores 0..7; optional NTFF profiling."""
    import concourse.bass_utils as bass_utils

    nc = _get_nc()
    tmpdir = None
    trace_cores = None
    if trace:
        trace = _ensure_ntff_hook()
    if trace:
        import tempfile
        tmpdir = tempfile.mkdtemp(prefix="mhsa_ntff_")
        _cache["trace_dir"] = tmpdir
        # no cloud creds in this container; keep artifacts local
        bass_utils.upload_artifacts = lambda d: f"local://{d}"
        if trace_all_cores:
            trace_cores = list(range(N_CORES))
    res = bass_utils.run_bass_kernel_spmd(
        nc, in_maps, list(range(N_CORES)), trace=trace, tmpdir=tmpdir,
        trace_cores=trace_cores)
    _cache["last_results"] = res
    return res


def kernel(x, W_Q, W_K, W_V, W_O):
    x = np.asarray(x, dtype=np.float32)
    in_maps = make_in_maps(x, np.asarray(W_Q, np.float32),
                           np.asarray(W_K, np.float32),
                           np.asarray(W_V, np.float32),
                           np.asarray(W_O, np.float32))
    trace = bool(int(os.environ.get("MHSA_TRACE", "0")))
    all_cores = bool(int(os.environ.get("MHSA_TRACE_ALL_CORES", "0")))
    res = run_on_cores(in_maps, trace=trace, trace_all_cores=all_cores)
    out = np.zeros((NT, D), dtype=np.float32)
    for r in res.results:
        out += np.asarray(r["out"], dtype=np.float32)
    return out.reshape(B, T, D)


# revision 26
# speedup vs baseline: 1.1799x; 1.1799x over previous
"""Multi-head self-attention (B=4, T=2048, D=1024, H=16) on 8 TRN2 NeuronCores.

Sharding: tensor-parallel over heads. Core c owns heads (2c, 2c+1):
  - W_Q/W_K/W_V rows [128c, 128c+128) -> per-core q/k/v of shape [T*B, 128]
  - causal attention for its 2 heads
  - partial output projection through W_O columns [128c, 128c+128)
Host sums the 8 partial outputs (the row-parallel W_O reduction).

Per-core schedule (batch-fused so engines stay dense):
  for b in 0..3:
    phase1(b): q/k/v projections for batch b, one 512-token chunk at a
      time (x streamed chunk-major, ONE dma per chunk).
    attn(b), per 512-q chunk: per 128-k-tile: h-packed score MMs (heads
      at array rows 0-63/64-127, concurrent) -> diag-mask add (DVE) ->
      exp (ACT, bf16 out) -> AV MMs accumulating [64 v-dims | ones] so
      row 64 is the softmax denominator. Diagonal k-tiles are column-
      sliced so fully-masked blocks are never computed.
      Chunk tail: 1/denom via DVE reciprocal_approx_fast (no ACT table
      swaps), GpSimd partition_broadcast, DVE mul into hoT, then the
      output projection + DMA for that chunk's 4 token-tiles.
  The dataflow scheduler overlaps attn(b) with phase1(b+1), keeping the
  PE dense (HAM stays at K=8/8) while ACT runs the exp stream.
"""

import os
import sys

import numpy as np

if "/opt/trn_rl_repo" not in sys.path:
    sys.path.insert(0, "/opt/trn_rl_repo")

import ml_dtypes

B, T, D, NH, DH = 4, 2048, 1024, 16, 64
NT = B * T          # 8192 tokens
MT = D // 128       # 8 model-dim tiles
NCH = NT // 512     # 16 token chunks
N_CORES = 8

_cache = {}


def _build_nc():
    from contextlib import ExitStack

    import concourse.mybir as mybir
    import concourse.tile as tile
    from concourse import bacc

    BF = mybir.dt.bfloat16
    F32 = mybir.dt.float32
    EXP = mybir.ActivationFunctionType.Exp
    LN = mybir.ActivationFunctionType.Ln

    nc = bacc.Bacc("TRN2", target_bir_lowering=False, debug=False)

    # x chunk-major: [chunk, partition, mt, col]
    xT_d = nc.dram_tensor("xT", [NCH, 128, MT, 512], BF, kind="ExternalInput")
    wq_d = nc.dram_tensor("wqT", [MT, 128, 128], BF, kind="ExternalInput")
    wk_d = nc.dram_tensor("wkT", [MT, 128, 128], BF, kind="ExternalInput")
    wv_d = nc.dram_tensor("wvT", [MT, 128, 128], BF, kind="ExternalInput")
    wo_d = nc.dram_tensor("woT", [128, D], BF, kind="ExternalInput")
    cm_d = nc.dram_tensor("cmask", [128, 2, 128], BF, kind="ExternalInput")
    out_d = nc.dram_tensor("out", [NT, D], BF, kind="ExternalOutput")

    with tile.TileContext(nc) as tc, ExitStack() as ctx:
        pers = ctx.enter_context(tc.tile_pool(name="pers", bufs=1))
        wq = pers.tile([128, MT, 128], BF)
        wk = pers.tile([128, MT, 128], BF)
        wv = pers.tile([128, MT, 128], BF)
        wo = pers.tile([128, D], BF)
        cmask = pers.tile([128, 2, 128], BF)

        P = ctx.enter_context
        xpool = P(tc.tile_pool(name="xc", bufs=4))
        qtp = P(tc.tile_pool(name="qt", bufs=3))
        ktp = P(tc.tile_pool(name="kt", bufs=3))
        vbp = P(tc.tile_pool(name="vbp", bufs=3))
        hop = P(tc.tile_pool(name="hop", bufs=3))
        exp_pool = P(tc.tile_pool(name="exp", bufs=6))
        uhp = P(tc.tile_pool(name="uhp", bufs=3))
        denp = P(tc.tile_pool(name="den", bufs=2))
        lnp = P(tc.tile_pool(name="lnp", bufs=2))
        invp = P(tc.tile_pool(name="inv", bufs=2))
        bcast_p = P(tc.tile_pool(name="bcast", bufs=2))
        rvp = P(tc.tile_pool(name="rvp", bufs=4))
        osbp = P(tc.tile_pool(name="osb", bufs=4))
        p1p = P(tc.tile_pool(name="p1", bufs=1, space="PSUM"))
        popp = P(tc.tile_pool(name="pop", bufs=1, space="PSUM"))
        spp = P(tc.tile_pool(name="sp", bufs=2, space="PSUM"))
        avp = P(tc.tile_pool(name="avp", bufs=1, space="PSUM"))

        if True:
            state = {"first": True}
            qkv = {}

            def phase1(b):
                # ---- phase 1: q/k/v projections for batch b ----
                first = state["first"]
                qt = qtp.tile([128, T], BF, tag="qt", name="qt")
                kt = ktp.tile([128, T], BF, tag="kt", name="kt")
                vb = vbp.tile([128, 16, 130], BF, tag="vb", name="vb")
                qkv[b] = (qt, kt, vb)
                for lc in range(4):
                    c = 4 * b + lc
                    cs = slice(lc * 512, (lc + 1) * 512)
                    if first:
                        # wq first (needed by the very first MM), then the
                        # first x chunk, then the rest of the weights
                        for mt in range(MT):
                            nc.sync.dma_start(out=wq[:, mt, :], in_=wq_d[mt])
                    xc = xpool.tile([128, MT, 512], BF, tag="xc", name="xc")
                    if first:
                        nc.sync.dma_start(out=xc[:, 0:4, :], in_=xT_d[c, :, 0:4, :])
                        nc.sync.dma_start(out=xc[:, 4:8, :], in_=xT_d[c, :, 4:8, :])
                    else:
                        nc.sync.dma_start(out=xc, in_=xT_d[c])
                    if first:
                        for mt in range(MT):
                            nc.sync.dma_start(out=wk[:, mt, :], in_=wk_d[mt])
                            nc.sync.dma_start(out=wv[:, mt, :], in_=wv_d[mt])
                        nc.sync.dma_start(out=wo[:], in_=wo_d[:])
                        nc.sync.dma_start(out=cmask, in_=cm_d[:])
                        first = state["first"] = False
                    pq = p1p.tile([128, 512], F32, tag="p1", name="pq")
                    for mt in range(MT):
                        nc.tensor.matmul(pq, wq[:, mt, :], xc[:, mt, :],
                                         start=(mt == 0), stop=(mt == MT - 1))
                    nc.vector.tensor_copy(out=qt[:, cs], in_=pq)
                    pk = p1p.tile([128, 512], F32, tag="p1", name="pk")
                    for mt in range(MT):
                        nc.tensor.matmul(pk, wk[:, mt, :], xc[:, mt, :],
                                         start=(mt == 0), stop=(mt == MT - 1))
                    nc.vector.tensor_copy(out=kt[:, cs], in_=pk)
                    pv = p1p.tile([128, 4, 128], F32, tag="p1", name="pv")
                    for tt in range(4):
                        for mt in range(MT):
                            nc.tensor.matmul(pv[:, tt, :],
                                             xc[:, mt, tt * 128:(tt + 1) * 128],
                                             wv[:, mt, :],
                                             start=(mt == 0), stop=(mt == MT - 1))
                    for h in range(2):
                        nc.vector.tensor_copy(
                            out=vb[:, lc * 4:(lc + 1) * 4, 65 * h:65 * h + 64],
                            in_=pv[:, :, 64 * h:64 * h + 64])
                # softmax-denominator ones columns
                nc.vector.memset(vb[:, :, 64:65], 1.0)
                nc.vector.memset(vb[:, :, 129:130], 1.0)

            def attn(b):
                # ---- phase 2: causal attention + projection for batch b ----
                qt, kt, vb = qkv.pop(b)
                hoT = hop.tile([128, T], BF, tag="hoT", name="hoT")
                uh_g = {}
                # last batch: singleton tail groups so the final
                # normalization+projection drains earlier
                groups = ((0, 1), (2, 3)) if b < B - 1 else ((0, 1), (2,), (3,))
                for grp in groups:
                    deng = denp.tile([128, 512], F32, tag="deng",
                                     name="deng")
                    nc.vector.memset(deng, 1.0)
                    for gi, qc in enumerate(grp):
                        q0 = qc * 512
                        nk = 4 * qc + 4
                        pavs = [avp.tile([65, 512], F32, tag=f"pav{h}",
                                         name=f"pav{h}") for h in range(2)]
                        for kt_i in range(nk):
                            off = 128 * (kt_i - 4 * qc) if kt_i >= 4 * qc else 0
                            pss = spp.tile([128, 2, 512], F32, tag="pss",
                                           name="pss")
                            for h in range(2):
                                hp = 64 * h
                                nc.tensor.matmul(
                                    pss[:, h, off:512],
                                    kt[hp:hp + 64, kt_i * 128:(kt_i + 1) * 128],
                                    qt[hp:hp + 64, q0 + off:q0 + 512],
                                    start=True, stop=True)
                            ex = exp_pool.tile([128, 2, 512], BF, tag="ex",
                                               name="ex")
                            nc.scalar.activation(out=ex[:, :, off:512],
                                                 in_=pss[:, :, off:512],
                                                 func=EXP, scale=0.125)
                            if kt_i >= 4 * qc:
                                nc.vector.tensor_mul(ex[:, :, off:off + 128],
                                                     ex[:, :, off:off + 128],
                                                     cmask)
                            for h in range(2):
                                nc.tensor.matmul(
                                    pavs[h][:, off:512],
                                    vb[:, kt_i, 65 * h:65 * h + 65],
                                    ex[:, h, off:512],
                                    start=(kt_i == 0), stop=(kt_i == nk - 1))
                        # chunk tail: stash unnormalized AV rows in SBUF
                        # (frees PSUM fast); gather denominators into
                        # 32-aligned partitions of the group tile so one
                        # ln+exp ACT pass covers the whole group.
                        uh = uhp.tile([128, 2, 512], BF, tag="uh", name="uh")
                        uh_g[qc] = uh
                        for h in range(2):
                            nc.vector.tensor_copy(out=uh[0:64, h, :],
                                                  in_=pavs[h][0:64, :])
                            r = 64 * gi + 32 * h
                            nc.vector.tensor_copy(out=deng[r:r + 1, :],
                                                  in_=pavs[h][64:65, :])
                    # group tail: 1/denoms, broadcast, normalize, project
                    lnd = lnp.tile([128, 512], F32, tag="lnd", name="lnd")
                    nc.scalar.activation(out=lnd, in_=deng, func=LN)
                    invg = invp.tile([128, 512], BF, tag="invg", name="invg")
                    with nc.allow_low_precision(
                            reason="softmax 1/denom via exp(-ln d)"):
                        nc.scalar.activation(out=invg, in_=lnd, func=EXP,
                                             scale=-1.0)
                    for gi, qc2 in enumerate(grp):
                        q2 = qc2 * 512
                        for h in range(2):
                            r = 64 * gi + 32 * h
                            # partition_broadcast only honors base
                            # partition 0 on HW: stage the row there
                            rv = rvp.tile([1, 512], BF, tag="rv", name="rv")
                            nc.vector.tensor_copy(out=rv,
                                                  in_=invg[r:r + 1, :])
                            invb = bcast_p.tile([64, 512], BF,
                                                tag=f"invb{h}",
                                                name=f"invb{h}")
                            nc.gpsimd.partition_broadcast(invb, rv)
                            nc.vector.tensor_mul(
                                hoT[64 * h:64 * h + 64, q2:q2 + 512],
                                uh_g[qc2][0:64, h, :], invb)
                    for qc2 in grp:
                        for tt in range(4 * qc2, 4 * qc2 + 4):
                            osb = osbp.tile([128, D], BF, tag="osb",
                                            name="osb")
                            for oc in range(2):
                                po = popp.tile([128, 512], F32, tag="po",
                                               name="po")
                                nc.tensor.matmul(
                                    po, hoT[:, tt * 128:(tt + 1) * 128],
                                    wo[:, oc * 512:(oc + 1) * 512],
                                    start=True, stop=True)
                                nc.vector.tensor_copy(
                                    out=osb[:, oc * 512:(oc + 1) * 512],
                                    in_=po)
                            to = b * T + tt * 128
                            nc.sync.dma_start(out=out_d[to:to + 128, :],
                                              in_=osb)

            # Emit phase1(b+1) BEFORE attn(b): Tile's per-engine sem
            # waits are tick thresholds, so a consumer of a DVE write
            # waits for every earlier-priority DVE op — next-batch q/k/v
            # casts must precede the previous batch's attention tail or
            # the whole batch serializes on DVE order.
            for b in range(B):
                phase1(b)
                attn(b)
    # Compile with the activation-table chooser steered to the one set
    # that contains BOTH exp and ln, so the kernel needs a single
    # ACT_TABLE_LOAD (the default per-function choice alternates between
    # exp_and_others and natural_log, reloading tables at every switch).
    # Set ids stay valid: the dict keeps its size and insertion order,
    # only the membership used for selection is narrowed.
    import concourse.bacc as bacc_mod
    orig_tables = bacc_mod.get_activation_tables

    def _steered_tables(arch):
        tabs = orig_tables(arch)
        keep = "natural_log_exp_and_others"
        if keep in tabs:
            tabs = {name: (fns if name == keep else fns - tabs[keep])
                    for name, fns in tabs.items()}
        return tabs

    bacc_mod.get_activation_tables = _steered_tables
    try:
        nc.compile()
    finally:
        bacc_mod.get_activation_tables = orig_tables
    return nc


def _get_nc():
    if "nc" not in _cache:
        _cache["nc"] = _build_nc()
    return _cache["nc"]


def _bf(a):
    return np.ascontiguousarray(a, dtype=np.float32).astype(ml_dtypes.bfloat16)


def make_in_maps(x, W_Q, W_K, W_V, W_O):
    xT = _bf(x.reshape(NT, D).T)                      # [D, NT]
    # [chunk, partition, mt, col]
    xTc = np.ascontiguousarray(
        xT.reshape(MT, 128, NCH, 512).transpose(2, 1, 0, 3))
    cmask = np.ones((128, 2, 128), dtype=np.float32)
    for kp in range(128):
        cmask[kp, :, :kp] = 0.0
    cmask = cmask.astype(ml_dtypes.bfloat16)
    in_maps = []
    for c in range(N_CORES):
        rs = slice(c * 128, (c + 1) * 128)
        in_maps.append({
            "xT": xTc,
            "wqT": _bf(W_Q[rs, :].T).reshape(MT, 128, 128),
            "wkT": _bf(W_K[rs, :].T).reshape(MT, 128, 128),
            "wvT": _bf(W_V[rs, :].T).reshape(MT, 128, 128),
            "woT": _bf(W_O[:, rs].T),
            "cmask": cmask,
        })
    return in_maps


def _ensure_ntff_hook():
    """Install antenv.axon_hooks shim (missing in this image) so
    run_bass_kernel_spmd(trace=True) can capture NTFF profiles."""
    try:
        from antenv import axon_hooks  # noqa: F401
        return True
    except ImportError:
        pass
    try:
        import contextlib
        import ctypes
        import types

        import antenv

        so_path = "/opt/axon/libaxon_pjrt.so"
        lib = ctypes.CDLL(so_path)
        if not hasattr(lib, "axon_start_nrt_profile"):
            return False
        lib.axon_start_nrt_profile.argtypes = [
            ctypes.POINTER(ctypes.c_int64), ctypes.c_size_t]
        lib.axon_start_nrt_profile.restype = ctypes.c_int64
        lib.axon_stop_nrt_profile.argtypes = [ctypes.c_char_p]
        lib.axon_stop_nrt_profile.restype = ctypes.c_int64

        @contextlib.contextmanager
        def _hook(output_dir, device_ids):
            import jax
            jax.devices()
            if device_ids:
                ids = (ctypes.c_int64 * len(device_ids))(*device_ids)
                rc = lib.axon_start_nrt_profile(ids, len(device_ids))
            else:
                rc = lib.axon_start_nrt_profile(None, 0)
            if rc != 0:
                raise RuntimeError(f"axon_start_nrt_profile rc={rc}")
            try:
                yield
            finally:
                n = lib.axon_stop_nrt_profile(str(output_dir).encode())
                print(f"ntff profile: {n} file(s) -> {output_dir}",
                      file=sys.stderr)

        mod = types.ModuleType("antenv.axon_hooks")
        mod.get_axon_ntff_profile_hook = lambda: _hook
        mod.set_axon_ntff_profile_hook = lambda h: None
        sys.modules["antenv.axon_hooks"] = mod
        antenv.axon_hooks = mod
        return True
    except Exception as e:  # pragma: no cover
        print(f"ntff hook install failed: {e}", file=sys.stderr)
        return False


def run_on_cores(in_maps, trace=False, trace_all_cores=False):
    """Compile once, run on cores 0..7; optional NTFF profiling."""
    import concourse.bass_utils as bass_utils

    nc = _get_nc()
    tmpdir = None
    trace_cores = None
    if trace:
        trace = _ensure_ntff_hook()
    if trace:
        import tempfile
        tmpdir = tempfile.mkdtemp(prefix="mhsa_ntff_")
        _cache["trace_dir"] = tmpdir
        # no cloud creds in this container; keep artifacts local
        bass_utils.upload_artifacts = lambda d: f"local://{d}"
        if trace_all_cores:
            trace_cores = list(range(N_CORES))
    res = bass_utils.run_bass_kernel_spmd(
        nc, in_maps, list(range(N_CORES)), trace=trace, tmpdir=tmpdir,
        trace_cores=trace_cores)
    _cache["last_results"] = res
    return res


def kernel(x, W_Q, W_K, W_V, W_O):
    x = np.asarray(x, dtype=np.float32)
    in_maps = make_in_maps(x, np.asarray(W_Q, np.float32),
                           np.asarray(W_K, np.float32),
                           np.asarray(W_V, np.float32),
                           np.asarray(W_O, np.float32))
    trace = bool(int(os.environ.get("MHSA_TRACE", "0")))
    all_cores = bool(int(os.environ.get("MHSA_TRACE_ALL_CORES", "0")))
    res = run_on_cores(in_maps, trace=trace, trace_all_cores=all_cores)
    out = np.zeros((NT, D), dtype=np.float32)
    for r in res.results:
        out += np.asarray(r["out"], dtype=np.float32)
    return out.reshape(B, T, D)
